# revision 62
# baseline (speedup 1.0000x reference)
"""Trainium2 Bass kernel for nn_CausalSelfAttention_14877766713804.

Full causal self-attention block (QKV proj + rmsnorm + rope + causal SDPA
with value-embedding mix + output proj), distributed over 8 NeuronCores as
(batch, head-group): core c handles batch c//2 and heads (c%2)*4..(c%2)*4+4.

Structure (v3):
  - x is shipped pre-transposed (and bf16) from the host, so the QKV
    matmuls need no on-chip transposes of x.
  - v = lam0*(x@Wv) + lam1*ve is formed entirely in PSUM: Wv is lam0-scaled
    on the host, ve is lam1-scaled bf16, and an identity matmul accumulates
    ve into the v PSUM group.
  - attention runs stripe-major (512-token q-stripes across all 4 local
    heads), interleaved with the QKV phase; each stripe's partial output
    projection feeds a pair ReduceScatter immediately, so only the last
    stripe's collective is exposed.
  - rmsnorm rstd = exp(-0.5*ln(mean+eps)) keeps the Activation engine on a
    single table set (Ln/Exp/Copy) - no table reloads.
  - the Pool engine runs ONLY the collectives: tile-framework deps are
    counting semaphores per engine, so any Pool work numbered after a
    collective would serialize on its completion.

Self-contained: hardcodes shapes from the problem spec.
"""
import numpy as np
import ml_dtypes

import concourse.bacc as bacc
import concourse.mybir as mybir
import concourse.tile as tile
from concourse.masks import make_identity
from concourse.bass_utils import run_bass_kernel_spmd

dt = mybir.dt
AF = mybir.ActivationFunctionType
ALU = mybir.AluOpType
BF16 = ml_dtypes.bfloat16

# Problem constants
B, T, DIM, H, HD = 4, 2048, 1024, 8, 128
HDIM = H * HD                     # 1024
ATTN_SCALE = 0.12
EPS = 1.1920929e-07               # np.finfo(np.float32).eps

N_CORES = 8
HG = 4                            # heads per core
F = HG * HD                       # 512 local qkv features per section
P = 128
HH = HD // 2
STRIPE = 512                      # q-stripe width in attention
TPS = STRIPE // P                 # token tiles per stripe
NEG_INF = -1.0e30


def _build_nc(t_len=T):
    n_tt = t_len // P             # token tiles
    n_st = t_len // STRIPE        # q stripes
    nc = bacc.Bacc(None, target_bir_lowering=False, num_devices=N_CORES)

    # ---- external I/O (per-core shards, host-prepped layouts) ----
    xt_d = nc.dram_tensor("xt", [P, n_tt, 8, P], dt.bfloat16, kind="ExternalInput")
    ve_d = nc.dram_tensor("ve", [t_len, F], dt.bfloat16, kind="ExternalInput")
    wq_d = nc.dram_tensor("wq", [P, 8, 3 * F], dt.bfloat16, kind="ExternalInput")
    cos_d = nc.dram_tensor("cosr", [P, n_tt, HD], dt.bfloat16, kind="ExternalInput")
    sin_d = nc.dram_tensor("sinr", [P, n_tt, HD], dt.bfloat16, kind="ExternalInput")
    msk_d = nc.dram_tensor("dmask", [P, P], dt.float32, kind="ExternalInput")
    wp_d = nc.dram_tensor("wp", [P, HG, DIM], dt.bfloat16, kind="ExternalInput")
    bias_d = nc.dram_tensor("biasx", [P, DIM], dt.float32, kind="ExternalInput")
    out_d = nc.dram_tensor("out", [t_len // 2, DIM], dt.float32,
                           kind="ExternalOutput")

    with tile.TileContext(nc) as tc:
        with (
            tc.tile_pool(name="const", bufs=1) as const,
            tc.tile_pool(name="dram", bufs=1, space="DRAM") as dram,
            tc.tile_pool(name="big", bufs=1) as big,
            tc.tile_pool(name="work", bufs=2) as work,
            tc.tile_pool(name="ptp", bufs=36) as ptp,
            tc.tile_pool(name="ps", bufs=2, space="PSUM") as ps,
        ):
            # ---- constants ----
            ident_bf = const.tile([P, P], dt.bfloat16)
            make_identity(nc, ident_bf)   # Pool, before any collective
            cos_sb = const.tile([P, n_tt, HD], dt.bfloat16)
            sin_sb = const.tile([P, n_tt, HD], dt.bfloat16)
            mask_sb = const.tile([P, P], dt.float32)
            bias_sb = const.tile([P, DIM], dt.float32)
            eps_sb = const.tile([P, 1], dt.float32)
            nc.vector.memset(eps_sb[:], EPS)
            magic_sb = const.tile([P, 1], dt.int32)
            nc.vector.memset(magic_sb[:], 0x5F3759DF)
            c15_sb = const.tile([P, 1], dt.float32)
            nc.vector.memset(c15_sb[:], 1.5)
            WP = const.tile([P, HG, DIM], dt.bfloat16)
            WT = const.tile([P, 8, 3 * F], dt.bfloat16)
            # rope tables on the scalar queue (needed only from stage_b(0),
            # well after the Act engine's first copies); weights in small
            # group-major chunks on the otherwise-idle Pool queue so QKV
            # matmul dc-slices land just ahead of their consumption
            nc.scalar.dma_start(cos_sb[:], cos_d[:])
            nc.scalar.dma_start(sin_sb[:], sin_d[:])
            for grp in range(3):
                for dc in range(8):
                    sl = slice(grp * F, (grp + 1) * F)
                    # v-group on the scalar queue (its Act work starts later);
                    # q/k on the idle Pool queue via SWDGE
                    eng = nc.scalar if grp == 2 else nc.gpsimd
                    eng.dma_start(WT[:, dc, sl], wq_d[:, dc, sl])
            nc.gpsimd.dma_start(mask_sb[:], msk_d[:])
            nc.gpsimd.dma_start(WP[:], wp_d[:])
            nc.gpsimd.dma_start(bias_sb[:], bias_d[:])

            # ---- persistent big tensors ----
            QT = big.tile([P, HG, t_len], dt.bfloat16)   # [hd, h, t] roped q
            KT = big.tile([P, HG, t_len], dt.bfloat16)
            Vb = big.tile([P, n_tt, HG * (HD + 1)], dt.bfloat16)
            # ones columns only (strided memset)
            vones = Vb[:].rearrange("p t (h c) -> p t h c", h=HG)[:, :, :, HD:HD + 1]
            nc.vector.memset(vones, 1.0)

            RSK = const.tile([P, n_tt, HG], dt.float32)   # ATTN_SCALE*rstd_k

            # collective buffers: per-stripe partials + reduce-scatter outs
            prt_ds = [dram.tile([STRIPE, DIM], dt.bfloat16, name=f"prt{s}")
                      for s in range(n_st)]
            rs_ds = [dram.tile([STRIPE // 2, DIM], dt.bfloat16, name=f"rs{s}")
                     for s in range(n_st)]

            # ============ phase 1: qkv + norm + rope + transposes ============
            def stage_a(tt):
                xT = work.tile([P, 8, P], dt.bfloat16, name="xT", tag="xT")
                nc.sync.dma_start(xT[:], xt_d[:, tt])
                ve_t = work.tile([P, F], dt.bfloat16, name="ve_t", tag="ve")
                nc.sync.dma_start(ve_t[:], ve_d[tt * P:(tt + 1) * P, :])
                q_t = work.tile([P, F], dt.bfloat16, name="q_t", tag="q")
                k_t = work.tile([P, F], dt.bfloat16, name="k_t", tag="k")
                for grp in range(3):
                    mm = ps.tile([P, F], dt.float32, name="mm", tag="acc")
                    for dc in range(8):
                        nc.tensor.matmul(
                            mm[:], xT[:, dc], WT[:, dc, grp * F:(grp + 1) * F],
                            start=(dc == 0), stop=(dc == 7 and grp != 2))
                    if grp == 0:
                        nc.scalar.copy(q_t[:], mm[:])
                    elif grp == 1:
                        nc.scalar.copy(k_t[:], mm[:])
                    else:
                        # v = lam0*(x@Wv) + lam1*ve  (both scales host-folded)
                        nc.tensor.matmul(mm[:], ident_bf[:], ve_t[:],
                                         start=False, stop=True)
                        nc.scalar.copy(
                            Vb[:, tt].rearrange("p (h c) -> p h c", h=HG)[:, :, 0:HD],
                            mm[:].rearrange("p (h c) -> p h c", h=HG))
                # rstd = 1/sqrt(mean(q^2)+eps): squares with fused accumulate
                # (DVE), then exp(-0.5*ln(mean+eps)) on Act (one table set)
                scr_q = work.tile([P, F], dt.bfloat16, name="scr_q", tag="scrq")
                scr_k = work.tile([P, F], dt.bfloat16, name="scr_k", tag="scrk")
                qss = work.tile([P, 2 * HG], dt.float32, name="qss", tag="qss")
                for h in range(HG):
                    hsl = slice(h * HD, (h + 1) * HD)
                    nc.vector.scalar_tensor_tensor(
                        out=scr_q[:, hsl], in0=q_t[:, hsl], scalar=1.0,
                        in1=q_t[:, hsl], op0=ALU.mult, op1=ALU.mult,
                        accum_out=qss[:, h:h + 1])
                    nc.vector.scalar_tensor_tensor(
                        out=scr_k[:, hsl], in0=k_t[:, hsl], scalar=1.0,
                        in1=k_t[:, hsl], op0=ALU.mult, op1=ALU.mult,
                        accum_out=qss[:, HG + h:HG + h + 1])
                rstd = work.tile([P, 2 * HG], dt.float32, name="rstd", tag="rstd")
                # rstd = (mean + eps)^(-0.5) on DVE via magic-number rsqrt +
                # one Newton step (keeps Act on the Exp/Copy table set with
                # zero table reloads; 0.2% max err, far below bf16 noise)
                mtmp = work.tile([P, 2 * HG], dt.float32, name="mtmp", tag="mtmp")
                nc.vector.scalar_tensor_tensor(
                    out=mtmp[:], in0=qss[:], scalar=1.0 / HD,
                    in1=eps_sb[:].broadcast_to([P, 2 * HG]),
                    op0=ALU.mult, op1=ALU.add)
                ish = work.tile([P, 2 * HG], dt.int32, name="ish", tag="ish")
                nc.vector.tensor_single_scalar(
                    ish[:], mtmp[:].bitcast(dt.int32), 1, ALU.arith_shift_right)
                nc.vector.tensor_tensor(
                    rstd[:].bitcast(dt.int32),
                    magic_sb[:].broadcast_to([P, 2 * HG]), ish[:], ALU.subtract)
                t2 = work.tile([P, 2 * HG], dt.float32, name="t2", tag="t2")
                nc.vector.tensor_tensor(t2[:], rstd[:], rstd[:], ALU.mult)
                nc.vector.tensor_tensor(t2[:], t2[:], mtmp[:], ALU.mult)
                nc.vector.scalar_tensor_tensor(
                    out=t2[:], in0=t2[:], scalar=-0.5,
                    in1=c15_sb[:].broadcast_to([P, 2 * HG]),
                    op0=ALU.mult, op1=ALU.add)
                # final Newton multiply lands in bf16 so the rope scale op
                # runs in the DVE 2x mode
                rstd_bf = work.tile([P, 2 * HG], dt.bfloat16,
                                    name="rstd_bf", tag="rstdb")
                nc.vector.tensor_tensor(rstd_bf[:], rstd[:], t2[:], ALU.mult)
                # k's rstd is applied inside the attention exp as a
                # per-partition scale (k tokens sit on partitions there), so
                # k skips its rope-side scale entirely. Use the Newton-
                # corrected rstd_bf, NOT the raw seed in `rstd`.
                nc.vector.tensor_scalar_mul(RSK[:, tt, :],
                                            rstd_bf[:, HG:2 * HG],
                                            ATTN_SCALE)
                return q_t, k_t, rstd_bf

            def stage_b(tt, q_t, k_t, rstd):
                cos3 = cos_sb[:, tt].unsqueeze(1).broadcast_to([P, HG, HD])
                sin3 = sin_sb[:, tt].unsqueeze(1).broadcast_to([P, HG, HD])
                for qk, src in enumerate((q_t, k_t)):
                    eng = nc.vector
                    s3 = src[:].rearrange("p (h c) -> p h c", h=HG)
                    if qk == 0:
                        rsb = (rstd[:, 0:HG]
                               .unsqueeze(2).broadcast_to([P, HG, HD]))
                        qs = work.tile([P, HG, HD], dt.bfloat16,
                                       name=f"qs{qk}", tag=f"qs{qk}")
                        eng.tensor_tensor(qs[:], s3, rsb, ALU.mult)
                    else:
                        qs = s3   # k stays unscaled; rstd_k rides the exp
                    r1 = work.tile([P, HG, HD], dt.bfloat16,
                                   name=f"r1_{qk}", tag=f"r1{qk}")
                    eng.tensor_tensor(r1[:, :, 0:HH], qs[:, :, HH:HD],
                                      sin3[:, :, 0:HH], ALU.mult)
                    eng.tensor_tensor(r1[:, :, HH:HD], qs[:, :, 0:HH],
                                      sin3[:, :, HH:HD], ALU.mult)
                    r2 = work.tile([P, HG, HD], dt.bfloat16,
                                   name=f"r2_{qk}", tag=f"r2{qk}")
                    eng.tensor_tensor(r2[:], qs[:], cos3, ALU.mult)
                    rr = work.tile([P, HG, HD], dt.bfloat16,
                                   name=f"rr{qk}", tag=f"rr{qk}")
                    nc.vector.tensor_add(rr[:], r1[:], r2[:])
                    dstT = QT if qk == 0 else KT
                    for h in range(HG):
                        tp = ps.tile([P, 8, P], dt.bfloat16, name="tp", tag="tp")
                        nc.tensor.transpose(tp[:, 0], rr[:, h], ident_bf[:])
                        dst = dstT[:, h, tt * P:(tt + 1) * P]
                        # early tiles run while Act is light and DVE is the
                        # binding engine; later tiles run under exp pressure
                        if tt < 8 or h % 2 == 0:
                            nc.scalar.copy(dst, tp[:, 0])
                        else:
                            nc.vector.tensor_copy(dst, tp[:, 0])

            # ============ phase 2: stripe-major causal attention ============
            def attn_stripe(s, tail_inline=False):
                Ys = work.tile([P, TPS, F], dt.bfloat16, name="Ys", tag="Y")
                prev = None
                for h in range(HG):
                    pts = []
                    for kt in range(TPS * s + TPS):
                        qoff = max(0, (kt - TPS * s) * P)
                        sp = ps.tile([P, STRIPE], dt.float32, name="sp", tag="sp")
                        nc.tensor.matmul(
                            sp[:, qoff:STRIPE],
                            KT[:, h, kt * P:(kt + 1) * P],
                            QT[:, h, s * STRIPE + qoff:(s + 1) * STRIPE],
                            start=True, stop=True)
                        if kt >= TPS * s:
                            # diagonal block: add causal -inf mask pre-exp
                            nc.vector.tensor_add(sp[:, qoff:qoff + P],
                                                 sp[:, qoff:qoff + P],
                                                 mask_sb[:])
                        pt = ptp.tile([P, STRIPE], dt.bfloat16,
                                      name="pt", tag="pt")
                        nc.scalar.activation(pt[:, qoff:STRIPE],
                                             sp[:, qoff:STRIPE],
                                             AF.Exp,
                                             scale=RSK[:, kt, h:h + 1])
                        pts.append(pt)
                    if prev is not None:
                        attn_pv(s, prev[0], prev[1], Ys)
                    prev = (h, pts)
                attn_pv(s, prev[0], prev[1], Ys,
                        tail_cb=(lambda j: tail_tile(s, Ys, j))
                        if tail_inline else None)
                if tail_inline:
                    issue_rs(s)
                return Ys

            def attn_pv(s, h, pts, Ys, tail_cb=None):
                for j in range(TPS):
                    jq = TPS * s + j
                    yt = ps.tile([P, F], dt.float32, name="yt", tag="yt")
                    for kt in range(jq + 1):
                        nc.tensor.matmul(
                            yt[:, 0:HD + 1],
                            pts[kt][:, j * P:(j + 1) * P],
                            Vb[:, kt, h * (HD + 1):(h + 1) * (HD + 1)],
                            start=(kt == 0), stop=(kt == jq))
                    rec = work.tile([P, 1], dt.float32, name="rec", tag="rec",
                                    bufs=4)
                    nc.vector.reciprocal(rec[:], yt[:, HD:HD + 1])
                    nc.vector.tensor_scalar_mul(
                        Ys[:, j, h * HD:(h + 1) * HD], yt[:, 0:HD], rec[:])
                    if tail_cb is not None:
                        # last head: emit this tile's projection right away so
                        # the final prt DMA (which gates the last RS) trails
                        # the last pv by one tile, not a whole stripe
                        tail_cb(j)

            def tail_tile(s, Ys, j):
                # late stripes run while Act is exp-saturated and DVE is
                # nearly idle, so their copies go to DVE; early ones split
                late = s >= 2
                ytiles = []
                for h in range(HG):
                    tp = ps.tile([P, 8, P], dt.bfloat16, name="typ", tag="tp")
                    nc.tensor.transpose(tp[:, 0], Ys[:, j, h * HD:(h + 1) * HD],
                                        ident_bf[:])
                    yT = work.tile([P, P], dt.bfloat16, name="yT", tag="yT",
                                   bufs=8)
                    if late or h % 2 == 0:
                        nc.vector.tensor_copy(yT[:], tp[:, 0])
                    else:
                        nc.scalar.copy(yT[:], tp[:, 0])
                    ytiles.append(yT)
                prt = work.tile([P, DIM], dt.bfloat16, name="prt", tag="prt")
                for half in range(2):
                    cp = ps.tile([P, F], dt.float32, name="cp", tag="acc")
                    for h in range(HG):
                        nc.tensor.matmul(
                            cp[:], ytiles[h][:],
                            WP[:, h, half * F:(half + 1) * F],
                            start=(h == 0), stop=(h == HG - 1))
                    if half == 1 or late:
                        nc.vector.tensor_copy(
                            prt[:, half * F:(half + 1) * F], cp[:])
                    else:
                        nc.scalar.copy(prt[:, 0:F], cp[:])
                nc.sync.dma_start(prt_ds[s][j * P:(j + 1) * P, :], prt[:])

            def issue_rs(s):
                # per-stripe pair ReduceScatter; Pool queue holds ONLY these
                nc.gpsimd.collective_compute(
                    "ReduceScatter", ALU.add,
                    replica_groups=[[i, i + 1] for i in range(0, N_CORES, 2)],
                    ins=[prt_ds[s].opt()], outs=[rs_ds[s].opt()])

            def stripe_tail(s, Ys):
                for j in range(TPS):
                    tail_tile(s, Ys, j)
                issue_rs(s)

            # phase 3 (per RS chunk): gather RS result + bias + output write.
            # Chunks are emitted two stripes after their collective is issued
            # so their queue slots never head-of-line-block on the collective.
            half_rows = STRIPE // 2          # 256 rows per core per stripe
            def phase3(s, last=False):
                for j in range(half_rows // P):
                    rs_sb = work.tile([P, DIM], dt.bfloat16,
                                      name="rs_sb", tag="rssb")
                    ld = nc.sync if (not last or j == 0) else nc.scalar
                    ld.dma_start(rs_sb[:], rs_ds[s][j * P:(j + 1) * P, :])
                    o_sb = work.tile([P, DIM], dt.float32, name="o_sb", tag="osb")
                    # for the final chunk, fan the two tiles across engine
                    # queues (Pool is past its last collective there)
                    if last and j == 1:
                        nc.gpsimd.tensor_add(o_sb[:], rs_sb[:], bias_sb[:])
                        st = nc.sync
                    else:
                        nc.vector.tensor_add(o_sb[:], rs_sb[:], bias_sb[:])
                        st = nc.scalar
                    st.dma_start(
                        out_d[s * half_rows + j * P:s * half_rows + (j + 1) * P, :],
                        o_sb[:])

            # ============ main interleaved schedule ============
            # stage_b runs one tile behind stage_a so the rstd chain's
            # latency hides behind the next tile's QKV matmuls
            Ys_prev = None
            pend = None

            def after_b(bt):
                nonlocal Ys_prev
                if bt % TPS == TPS - 1:
                    s = bt // TPS
                    if s >= 1:
                        stripe_tail(s - 1, Ys_prev)
                    Ys_prev = attn_stripe(s, tail_inline=(s == n_st - 1))

            for tt in range(n_tt):
                cur = (tt, *stage_a(tt))
                if pend is not None:
                    stage_b(*pend)
                    after_b(pend[0])
                pend = cur
            stage_b(*pend)
            after_b(pend[0])
            # pin each phase-3 chunk just past its collective's completion:
            # a greedy earlier placement head-of-line-blocks the Act/DVE
            # queues on the collective; chunks 0-2 then fill the RS3 window
            for s in range(n_st):
                with tc.tile_wait_until(0.235):
                    phase3(s, last=(s == n_st - 1))

    nc.compile()
    return nc


_NC_CACHE = {}


def _get_nc(t_len=T):
    if t_len not in _NC_CACHE:
        _NC_CACHE[t_len] = _build_nc(t_len)
    return _NC_CACHE[t_len]


def make_in_maps(x, ve, qkv_w, lambdas, c_proj_w, c_proj_b, t_len=T):
    """Host-side sharding + layout prep (relayout/slicing + dtype casts)."""
    x = np.asarray(x, np.float32)
    ve = np.asarray(ve, np.float32)
    qkv_w = np.asarray(qkv_w, np.float32)
    lambdas = np.asarray(lambdas, np.float32)
    c_proj_w = np.asarray(c_proj_w, np.float32)
    c_proj_b = np.asarray(c_proj_b, np.float32)

    n_tt = t_len // P
    half = HD // 2
    inv_freq = (1.0 / (10000.0 ** (np.arange(half, dtype=np.float64) / half)))
    ang = np.arange(t_len, dtype=np.float64)[:, None] * inv_freq[None, :]
    cos = np.cos(ang).astype(np.float32)
    sin = np.sin(ang).astype(np.float32)
    cosr = np.concatenate([cos, cos], axis=1)            # [T, 128]
    sinr = np.concatenate([-sin, sin], axis=1)           # [T, 128]
    cosr = np.ascontiguousarray(
        cosr.reshape(n_tt, P, HD).transpose(1, 0, 2)).astype(BF16)
    sinr = np.ascontiguousarray(
        sinr.reshape(n_tt, P, HD).transpose(1, 0, 2)).astype(BF16)

    kk, qq = np.meshgrid(np.arange(P), np.arange(P), indexing="ij")
    dmask = np.where(kk <= qq, 0.0, NEG_INF).astype(np.float32)
    biasx = np.tile(c_proj_b.reshape(1, DIM), (P, 1)).astype(np.float32)

    in_maps = []
    for c in range(N_CORES):
        b, hg = c // 2, c % 2
        # fused qkv weight slice for this head group; v rows lam0-scaled
        wslc = np.concatenate(
            [qkv_w[e, hg * F:(hg + 1) * F, :] for e in range(3)], axis=0)
        wslc = wslc.copy()
        wslc[2 * F:] *= lambdas[0]
        wq = np.ascontiguousarray(
            wslc.T.reshape(8, P, 3 * F).transpose(1, 0, 2)).astype(BF16)
        # x pre-transposed: [dim, t] -> [128, n_tt, 8, 128]
        xt = np.ascontiguousarray(
            x[b].T.reshape(8, P, n_tt, P).transpose(1, 2, 0, 3)).astype(BF16)
        vesl = np.ascontiguousarray(
            ve[b].reshape(t_len, H, HD)[:, hg * HG:(hg + 1) * HG, :]
            .reshape(t_len, F) * lambdas[1]).astype(BF16)
        # per-local-head projection weights [hd, h, dim_out]
        wp = np.stack(
            [c_proj_w[:, (hg * HG + h) * HD:(hg * HG + h + 1) * HD].T
             for h in range(HG)], axis=1).astype(BF16)
        in_maps.append({
            "xt": xt,
            "ve": vesl,
            "wq": wq,
            "cosr": cosr,
            "sinr": sinr,
            "dmask": dmask,
            "wp": np.ascontiguousarray(wp),
            "biasx": biasx,
        })
    return in_maps


def assemble(results):
    """Reassemble per-core ReduceScatter shards into [B, T, DIM]."""
    hr = STRIPE // 2
    out = np.zeros((B, T, DIM), np.float32)
    for b in range(B):
        r0 = np.asarray(results[2 * b]["out"])
        r1 = np.asarray(results[2 * b + 1]["out"])
        for s in range(T // STRIPE):
            out[b, s * STRIPE:s * STRIPE + hr] = r0[s * hr:(s + 1) * hr]
            out[b, s * STRIPE + hr:(s + 1) * STRIPE] = r1[s * hr:(s + 1) * hr]
    return out


def kernel(x, ve, qkv_w, lambdas, c_proj_w, c_proj_b):
    nc = _get_nc(T)
    in_maps = make_in_maps(x, ve, qkv_w, lambdas, c_proj_w, c_proj_b, T)
    r = run_bass_kernel_spmd(nc, in_maps, list(range(N_CORES)))
    return assemble(r.results)


# revision 63
# speedup vs baseline: 1.0179x; 1.0179x over previous
"""Trainium2 Bass kernel for nn_CausalSelfAttention_14877766713804.

Full causal self-attention block (QKV proj + rmsnorm + rope + causal SDPA
with value-embedding mix + output proj), distributed over 8 NeuronCores as
(batch, head-group): core c handles batch c//2 and heads (c%2)*4..(c%2)*4+4.

Structure (v3):
  - x is shipped pre-transposed (and bf16) from the host, so the QKV
    matmuls need no on-chip transposes of x.
  - v = lam0*(x@Wv) + lam1*ve is formed entirely in PSUM: Wv is lam0-scaled
    on the host, ve is lam1-scaled bf16, and an identity matmul accumulates
    ve into the v PSUM group.
  - attention runs stripe-major (512-token q-stripes across all 4 local
    heads), interleaved with the QKV phase; each stripe's partial output
    projection feeds a pair ReduceScatter immediately, so only the last
    stripe's collective is exposed.
  - rmsnorm rstd = exp(-0.5*ln(mean+eps)) keeps the Activation engine on a
    single table set (Ln/Exp/Copy) - no table reloads.
  - the Pool engine runs ONLY the collectives: tile-framework deps are
    counting semaphores per engine, so any Pool work numbered after a
    collective would serialize on its completion.

Self-contained: hardcodes shapes from the problem spec.
"""
import numpy as np
import ml_dtypes

import concourse.bacc as bacc
import concourse.mybir as mybir
import concourse.tile as tile
from concourse.masks import make_identity
from concourse.bass_utils import run_bass_kernel_spmd

dt = mybir.dt
AF = mybir.ActivationFunctionType
ALU = mybir.AluOpType
BF16 = ml_dtypes.bfloat16

# Problem constants
B, T, DIM, H, HD = 4, 2048, 1024, 8, 128
HDIM = H * HD                     # 1024
ATTN_SCALE = 0.12
EPS = 1.1920929e-07               # np.finfo(np.float32).eps

N_CORES = 8
HG = 4                            # heads per core
F = HG * HD                       # 512 local qkv features per section
P = 128
HH = HD // 2
STRIPE = 512                      # q-stripe width in attention
TPS = STRIPE // P                 # token tiles per stripe
NEG_INF = -1.0e30


def _build_nc(t_len=T):
    n_tt = t_len // P             # token tiles
    n_st = t_len // STRIPE        # q stripes
    nc = bacc.Bacc(None, target_bir_lowering=False, num_devices=N_CORES)

    # ---- external I/O (per-core shards, host-prepped layouts) ----
    xt_d = nc.dram_tensor("xt", [P, n_tt, 8, P], dt.bfloat16, kind="ExternalInput")
    ve_d = nc.dram_tensor("ve", [t_len, F], dt.bfloat16, kind="ExternalInput")
    wq_d = nc.dram_tensor("wq", [P, 8, 3 * F], dt.bfloat16, kind="ExternalInput")
    cos_d = nc.dram_tensor("cosr", [P, n_tt, HD], dt.bfloat16, kind="ExternalInput")
    sin_d = nc.dram_tensor("sinr", [P, n_tt, HD], dt.bfloat16, kind="ExternalInput")
    msk_d = nc.dram_tensor("dmask", [P, P], dt.float32, kind="ExternalInput")
    wp_d = nc.dram_tensor("wp", [P, HG, DIM], dt.bfloat16, kind="ExternalInput")
    bias_d = nc.dram_tensor("biasx", [P, DIM], dt.bfloat16, kind="ExternalInput")
    out_d = nc.dram_tensor("out", [t_len // 2, DIM], dt.bfloat16,
                           kind="ExternalOutput")

    with tile.TileContext(nc) as tc:
        with (
            tc.tile_pool(name="const", bufs=1) as const,
            tc.tile_pool(name="dram", bufs=1, space="DRAM") as dram,
            tc.tile_pool(name="big", bufs=1) as big,
            tc.tile_pool(name="work", bufs=2) as work,
            tc.tile_pool(name="ptp", bufs=36) as ptp,
            tc.tile_pool(name="ps", bufs=2, space="PSUM") as ps,
        ):
            # ---- constants ----
            ident_bf = const.tile([P, P], dt.bfloat16)
            make_identity(nc, ident_bf)   # Pool, before any collective
            cos_sb = const.tile([P, n_tt, HD], dt.bfloat16)
            sin_sb = const.tile([P, n_tt, HD], dt.bfloat16)
            mask_sb = const.tile([P, P], dt.float32)
            bias_sb = const.tile([P, DIM], dt.bfloat16)
            eps_sb = const.tile([P, 1], dt.float32)
            nc.vector.memset(eps_sb[:], EPS)
            magic_sb = const.tile([P, 1], dt.int32)
            nc.vector.memset(magic_sb[:], 0x5F3759DF)
            c15_sb = const.tile([P, 1], dt.float32)
            nc.vector.memset(c15_sb[:], 1.5)
            WP = const.tile([P, HG, DIM], dt.bfloat16)
            WT = const.tile([P, 8, 3 * F], dt.bfloat16)
            # rope tables on the scalar queue (needed only from stage_b(0),
            # well after the Act engine's first copies); weights in small
            # group-major chunks on the otherwise-idle Pool queue so QKV
            # matmul dc-slices land just ahead of their consumption
            nc.scalar.dma_start(cos_sb[:], cos_d[:])
            nc.scalar.dma_start(sin_sb[:], sin_d[:])
            for grp in range(3):
                for dc in range(8):
                    sl = slice(grp * F, (grp + 1) * F)
                    # v-group on the scalar queue (its Act work starts later);
                    # q/k on the idle Pool queue via SWDGE
                    eng = nc.scalar if grp == 2 else nc.gpsimd
                    eng.dma_start(WT[:, dc, sl], wq_d[:, dc, sl])
            nc.gpsimd.dma_start(mask_sb[:], msk_d[:])
            nc.gpsimd.dma_start(WP[:], wp_d[:])
            nc.gpsimd.dma_start(bias_sb[:], bias_d[:])

            # ---- persistent big tensors ----
            QT = big.tile([P, HG, t_len], dt.bfloat16)   # [hd, h, t] roped q
            KT = big.tile([P, HG, t_len], dt.bfloat16)
            Vb = big.tile([P, n_tt, HG * (HD + 1)], dt.bfloat16)
            # ones columns only (strided memset)
            vones = Vb[:].rearrange("p t (h c) -> p t h c", h=HG)[:, :, :, HD:HD + 1]
            nc.vector.memset(vones, 1.0)

            RSK = const.tile([P, n_tt, HG], dt.float32)   # ATTN_SCALE*rstd_k

            # collective buffers: per-stripe partials + reduce-scatter outs
            prt_ds = [dram.tile([STRIPE, DIM], dt.bfloat16, name=f"prt{s}")
                      for s in range(n_st)]
            rs_ds = [dram.tile([STRIPE // 2, DIM], dt.bfloat16, name=f"rs{s}")
                     for s in range(n_st)]

            # ============ phase 1: qkv + norm + rope + transposes ============
            def stage_a(tt):
                xT = work.tile([P, 8, P], dt.bfloat16, name="xT", tag="xT")
                nc.sync.dma_start(xT[:], xt_d[:, tt])
                ve_t = work.tile([P, F], dt.bfloat16, name="ve_t", tag="ve")
                nc.sync.dma_start(ve_t[:], ve_d[tt * P:(tt + 1) * P, :])
                q_t = work.tile([P, F], dt.bfloat16, name="q_t", tag="q")
                k_t = work.tile([P, F], dt.bfloat16, name="k_t", tag="k")
                for grp in range(3):
                    mm = ps.tile([P, F], dt.float32, name="mm", tag="acc")
                    for dc in range(8):
                        nc.tensor.matmul(
                            mm[:], xT[:, dc], WT[:, dc, grp * F:(grp + 1) * F],
                            start=(dc == 0), stop=(dc == 7 and grp != 2))
                    if grp == 0:
                        nc.scalar.copy(q_t[:], mm[:])
                    elif grp == 1:
                        nc.scalar.copy(k_t[:], mm[:])
                    else:
                        # v = lam0*(x@Wv) + lam1*ve  (both scales host-folded)
                        nc.tensor.matmul(mm[:], ident_bf[:], ve_t[:],
                                         start=False, stop=True)
                        nc.scalar.copy(
                            Vb[:, tt].rearrange("p (h c) -> p h c", h=HG)[:, :, 0:HD],
                            mm[:].rearrange("p (h c) -> p h c", h=HG))
                # rstd = 1/sqrt(mean(q^2)+eps): squares with fused accumulate
                # (DVE), then exp(-0.5*ln(mean+eps)) on Act (one table set)
                scr_q = work.tile([P, F], dt.bfloat16, name="scr_q", tag="scrq")
                scr_k = work.tile([P, F], dt.bfloat16, name="scr_k", tag="scrk")
                qss = work.tile([P, 2 * HG], dt.float32, name="qss", tag="qss")
                for h in range(HG):
                    hsl = slice(h * HD, (h + 1) * HD)
                    nc.vector.scalar_tensor_tensor(
                        out=scr_q[:, hsl], in0=q_t[:, hsl], scalar=1.0,
                        in1=q_t[:, hsl], op0=ALU.mult, op1=ALU.mult,
                        accum_out=qss[:, h:h + 1])
                    nc.vector.scalar_tensor_tensor(
                        out=scr_k[:, hsl], in0=k_t[:, hsl], scalar=1.0,
                        in1=k_t[:, hsl], op0=ALU.mult, op1=ALU.mult,
                        accum_out=qss[:, HG + h:HG + h + 1])
                rstd = work.tile([P, 2 * HG], dt.float32, name="rstd", tag="rstd")
                # rstd = (mean + eps)^(-0.5) on DVE via magic-number rsqrt +
                # one Newton step (keeps Act on the Exp/Copy table set with
                # zero table reloads; 0.2% max err, far below bf16 noise)
                mtmp = work.tile([P, 2 * HG], dt.float32, name="mtmp", tag="mtmp")
                nc.vector.scalar_tensor_tensor(
                    out=mtmp[:], in0=qss[:], scalar=1.0 / HD,
                    in1=eps_sb[:].broadcast_to([P, 2 * HG]),
                    op0=ALU.mult, op1=ALU.add)
                ish = work.tile([P, 2 * HG], dt.int32, name="ish", tag="ish")
                nc.vector.tensor_single_scalar(
                    ish[:], mtmp[:].bitcast(dt.int32), 1, ALU.arith_shift_right)
                nc.vector.tensor_tensor(
                    rstd[:].bitcast(dt.int32),
                    magic_sb[:].broadcast_to([P, 2 * HG]), ish[:], ALU.subtract)
                t2 = work.tile([P, 2 * HG], dt.float32, name="t2", tag="t2")
                nc.vector.tensor_tensor(t2[:], rstd[:], rstd[:], ALU.mult)
                nc.vector.tensor_tensor(t2[:], t2[:], mtmp[:], ALU.mult)
                nc.vector.scalar_tensor_tensor(
                    out=t2[:], in0=t2[:], scalar=-0.5,
                    in1=c15_sb[:].broadcast_to([P, 2 * HG]),
                    op0=ALU.mult, op1=ALU.add)
                # final Newton multiply lands in bf16 so the rope scale op
                # runs in the DVE 2x mode
                rstd_bf = work.tile([P, 2 * HG], dt.bfloat16,
                                    name="rstd_bf", tag="rstdb")
                nc.vector.tensor_tensor(rstd_bf[:], rstd[:], t2[:], ALU.mult)
                # k's rstd is applied inside the attention exp as a
                # per-partition scale (k tokens sit on partitions there), so
                # k skips its rope-side scale entirely. Use the Newton-
                # corrected rstd_bf, NOT the raw seed in `rstd`.
                nc.vector.tensor_scalar_mul(RSK[:, tt, :],
                                            rstd_bf[:, HG:2 * HG],
                                            ATTN_SCALE)
                return q_t, k_t, rstd_bf

            def stage_b(tt, q_t, k_t, rstd):
                cos3 = cos_sb[:, tt].unsqueeze(1).broadcast_to([P, HG, HD])
                sin3 = sin_sb[:, tt].unsqueeze(1).broadcast_to([P, HG, HD])
                for qk, src in enumerate((q_t, k_t)):
                    eng = nc.vector
                    s3 = src[:].rearrange("p (h c) -> p h c", h=HG)
                    if qk == 0:
                        rsb = (rstd[:, 0:HG]
                               .unsqueeze(2).broadcast_to([P, HG, HD]))
                        qs = work.tile([P, HG, HD], dt.bfloat16,
                                       name=f"qs{qk}", tag=f"qs{qk}")
                        eng.tensor_tensor(qs[:], s3, rsb, ALU.mult)
                    else:
                        qs = s3   # k stays unscaled; rstd_k rides the exp
                    r1 = work.tile([P, HG, HD], dt.bfloat16,
                                   name=f"r1_{qk}", tag=f"r1{qk}")
                    eng.tensor_tensor(r1[:, :, 0:HH], qs[:, :, HH:HD],
                                      sin3[:, :, 0:HH], ALU.mult)
                    eng.tensor_tensor(r1[:, :, HH:HD], qs[:, :, 0:HH],
                                      sin3[:, :, HH:HD], ALU.mult)
                    r2 = work.tile([P, HG, HD], dt.bfloat16,
                                   name=f"r2_{qk}", tag=f"r2{qk}")
                    eng.tensor_tensor(r2[:], qs[:], cos3, ALU.mult)
                    rr = work.tile([P, HG, HD], dt.bfloat16,
                                   name=f"rr{qk}", tag=f"rr{qk}")
                    nc.vector.tensor_add(rr[:], r1[:], r2[:])
                    dstT = QT if qk == 0 else KT
                    for h in range(HG):
                        tp = ps.tile([P, 8, P], dt.bfloat16, name="tp", tag="tp")
                        nc.tensor.transpose(tp[:, 0], rr[:, h], ident_bf[:])
                        dst = dstT[:, h, tt * P:(tt + 1) * P]
                        # early tiles run while Act is light and DVE is the
                        # binding engine; later tiles run under exp pressure
                        if tt < 8 or h % 2 == 0:
                            nc.scalar.copy(dst, tp[:, 0])
                        else:
                            nc.vector.tensor_copy(dst, tp[:, 0])

            # ============ phase 2: stripe-major causal attention ============
            def attn_stripe(s, tail_inline=False):
                Ys = work.tile([P, TPS, F], dt.bfloat16, name="Ys", tag="Y")
                prev = None
                for h in range(HG):
                    pts = []
                    for kt in range(TPS * s + TPS):
                        qoff = max(0, (kt - TPS * s) * P)
                        sp = ps.tile([P, STRIPE], dt.float32, name="sp", tag="sp")
                        nc.tensor.matmul(
                            sp[:, qoff:STRIPE],
                            KT[:, h, kt * P:(kt + 1) * P],
                            QT[:, h, s * STRIPE + qoff:(s + 1) * STRIPE],
                            start=True, stop=True)
                        if kt >= TPS * s:
                            # diagonal block: add causal -inf mask pre-exp
                            nc.vector.tensor_add(sp[:, qoff:qoff + P],
                                                 sp[:, qoff:qoff + P],
                                                 mask_sb[:])
                        pt = ptp.tile([P, STRIPE], dt.bfloat16,
                                      name="pt", tag="pt")
                        nc.scalar.activation(pt[:, qoff:STRIPE],
                                             sp[:, qoff:STRIPE],
                                             AF.Exp,
                                             scale=RSK[:, kt, h:h + 1])
                        pts.append(pt)
                    if prev is not None:
                        attn_pv(s, prev[0], prev[1], Ys)
                    prev = (h, pts)
                attn_pv(s, prev[0], prev[1], Ys,
                        tail_cb=(lambda j: tail_tile(s, Ys, j))
                        if tail_inline else None)
                if tail_inline:
                    issue_rs(s)
                return Ys

            def attn_pv(s, h, pts, Ys, tail_cb=None):
                for j in range(TPS):
                    jq = TPS * s + j
                    yt = ps.tile([P, F], dt.float32, name="yt", tag="yt")
                    for kt in range(jq + 1):
                        nc.tensor.matmul(
                            yt[:, 0:HD + 1],
                            pts[kt][:, j * P:(j + 1) * P],
                            Vb[:, kt, h * (HD + 1):(h + 1) * (HD + 1)],
                            start=(kt == 0), stop=(kt == jq))
                    rec = work.tile([P, 1], dt.float32, name="rec", tag="rec",
                                    bufs=4)
                    nc.vector.reciprocal(rec[:], yt[:, HD:HD + 1])
                    nc.vector.tensor_scalar_mul(
                        Ys[:, j, h * HD:(h + 1) * HD], yt[:, 0:HD], rec[:])
                    if tail_cb is not None:
                        # last head: emit this tile's projection right away so
                        # the final prt DMA (which gates the last RS) trails
                        # the last pv by one tile, not a whole stripe
                        tail_cb(j)

            def tail_tile(s, Ys, j):
                # late stripes run while Act is exp-saturated and DVE is
                # nearly idle, so their copies go to DVE; early ones split
                late = s >= 2
                ytiles = []
                for h in range(HG):
                    tp = ps.tile([P, 8, P], dt.bfloat16, name="typ", tag="tp")
                    nc.tensor.transpose(tp[:, 0], Ys[:, j, h * HD:(h + 1) * HD],
                                        ident_bf[:])
                    yT = work.tile([P, P], dt.bfloat16, name="yT", tag="yT",
                                   bufs=8)
                    if late or h % 2 == 0:
                        nc.vector.tensor_copy(yT[:], tp[:, 0])
                    else:
                        nc.scalar.copy(yT[:], tp[:, 0])
                    ytiles.append(yT)
                prt = work.tile([P, DIM], dt.bfloat16, name="prt", tag="prt")
                for half in range(2):
                    cp = ps.tile([P, F], dt.float32, name="cp", tag="acc")
                    for h in range(HG):
                        nc.tensor.matmul(
                            cp[:], ytiles[h][:],
                            WP[:, h, half * F:(half + 1) * F],
                            start=(h == 0), stop=(h == HG - 1))
                    if half == 1 or late:
                        nc.vector.tensor_copy(
                            prt[:, half * F:(half + 1) * F], cp[:])
                    else:
                        nc.scalar.copy(prt[:, 0:F], cp[:])
                nc.sync.dma_start(prt_ds[s][j * P:(j + 1) * P, :], prt[:])

            def issue_rs(s):
                # per-stripe pair ReduceScatter; Pool queue holds ONLY these
                nc.gpsimd.collective_compute(
                    "ReduceScatter", ALU.add,
                    replica_groups=[[i, i + 1] for i in range(0, N_CORES, 2)],
                    ins=[prt_ds[s].opt()], outs=[rs_ds[s].opt()])

            def stripe_tail(s, Ys):
                for j in range(TPS):
                    tail_tile(s, Ys, j)
                issue_rs(s)

            # phase 3 (per RS chunk): gather RS result + bias + output write.
            # Chunks are emitted two stripes after their collective is issued
            # so their queue slots never head-of-line-block on the collective.
            half_rows = STRIPE // 2          # 256 rows per core per stripe
            def phase3(s, last=False):
                for j in range(half_rows // P):
                    rs_sb = work.tile([P, DIM], dt.bfloat16,
                                      name="rs_sb", tag="rssb")
                    ld = nc.sync if (not last or j == 0) else nc.scalar
                    ld.dma_start(rs_sb[:], rs_ds[s][j * P:(j + 1) * P, :])
                    o_sb = work.tile([P, DIM], dt.bfloat16, name="o_sb", tag="osb")
                    # for the final chunk, fan the two tiles across engine
                    # queues (Pool is past its last collective there)
                    if last and j == 1:
                        nc.gpsimd.tensor_add(o_sb[:], rs_sb[:], bias_sb[:])
                        st = nc.sync
                    else:
                        nc.vector.tensor_add(o_sb[:], rs_sb[:], bias_sb[:])
                        st = nc.scalar
                    st.dma_start(
                        out_d[s * half_rows + j * P:s * half_rows + (j + 1) * P, :],
                        o_sb[:])

            # ============ main interleaved schedule ============
            # stage_b runs one tile behind stage_a so the rstd chain's
            # latency hides behind the next tile's QKV matmuls
            Ys_prev = None
            pend = None

            def after_b(bt):
                nonlocal Ys_prev
                if bt % TPS == TPS - 1:
                    s = bt // TPS
                    if s >= 1:
                        stripe_tail(s - 1, Ys_prev)
                    Ys_prev = attn_stripe(s, tail_inline=(s == n_st - 1))

            for tt in range(n_tt):
                cur = (tt, *stage_a(tt))
                if pend is not None:
                    stage_b(*pend)
                    after_b(pend[0])
                pend = cur
            stage_b(*pend)
            after_b(pend[0])
            # pin each phase-3 chunk just past its collective's completion:
            # a greedy earlier placement head-of-line-blocks the Act/DVE
            # queues on the collective; chunks 0-2 then fill the RS3 window
            for s in range(n_st):
                with tc.tile_wait_until(0.235):
                    phase3(s, last=(s == n_st - 1))

    nc.compile()
    return nc


_NC_CACHE = {}


def _get_nc(t_len=T):
    if t_len not in _NC_CACHE:
        _NC_CACHE[t_len] = _build_nc(t_len)
    return _NC_CACHE[t_len]


def make_in_maps(x, ve, qkv_w, lambdas, c_proj_w, c_proj_b, t_len=T):
    """Host-side sharding + layout prep (relayout/slicing + dtype casts)."""
    x = np.asarray(x, np.float32)
    ve = np.asarray(ve, np.float32)
    qkv_w = np.asarray(qkv_w, np.float32)
    lambdas = np.asarray(lambdas, np.float32)
    c_proj_w = np.asarray(c_proj_w, np.float32)
    c_proj_b = np.asarray(c_proj_b, np.float32)

    n_tt = t_len // P
    half = HD // 2
    inv_freq = (1.0 / (10000.0 ** (np.arange(half, dtype=np.float64) / half)))
    ang = np.arange(t_len, dtype=np.float64)[:, None] * inv_freq[None, :]
    cos = np.cos(ang).astype(np.float32)
    sin = np.sin(ang).astype(np.float32)
    cosr = np.concatenate([cos, cos], axis=1)            # [T, 128]
    sinr = np.concatenate([-sin, sin], axis=1)           # [T, 128]
    cosr = np.ascontiguousarray(
        cosr.reshape(n_tt, P, HD).transpose(1, 0, 2)).astype(BF16)
    sinr = np.ascontiguousarray(
        sinr.reshape(n_tt, P, HD).transpose(1, 0, 2)).astype(BF16)

    kk, qq = np.meshgrid(np.arange(P), np.arange(P), indexing="ij")
    dmask = np.where(kk <= qq, 0.0, NEG_INF).astype(np.float32)
    biasx = np.tile(c_proj_b.reshape(1, DIM), (P, 1)).astype(BF16)

    in_maps = []
    for c in range(N_CORES):
        b, hg = c // 2, c % 2
        # fused qkv weight slice for this head group; v rows lam0-scaled
        wslc = np.concatenate(
            [qkv_w[e, hg * F:(hg + 1) * F, :] for e in range(3)], axis=0)
        wslc = wslc.copy()
        wslc[2 * F:] *= lambdas[0]
        wq = np.ascontiguousarray(
            wslc.T.reshape(8, P, 3 * F).transpose(1, 0, 2)).astype(BF16)
        # x pre-transposed: [dim, t] -> [128, n_tt, 8, 128]
        xt = np.ascontiguousarray(
            x[b].T.reshape(8, P, n_tt, P).transpose(1, 2, 0, 3)).astype(BF16)
        vesl = np.ascontiguousarray(
            ve[b].reshape(t_len, H, HD)[:, hg * HG:(hg + 1) * HG, :]
            .reshape(t_len, F) * lambdas[1]).astype(BF16)
        # per-local-head projection weights [hd, h, dim_out]
        wp = np.stack(
            [c_proj_w[:, (hg * HG + h) * HD:(hg * HG + h + 1) * HD].T
             for h in range(HG)], axis=1).astype(BF16)
        in_maps.append({
            "xt": xt,
            "ve": vesl,
            "wq": wq,
            "cosr": cosr,
            "sinr": sinr,
            "dmask": dmask,
            "wp": np.ascontiguousarray(wp),
            "biasx": biasx,
        })
    return in_maps


def assemble(results):
    """Reassemble per-core ReduceScatter shards into [B, T, DIM]."""
    hr = STRIPE // 2
    out = np.zeros((B, T, DIM), np.float32)
    for b in range(B):
        r0 = np.asarray(results[2 * b]["out"])
        r1 = np.asarray(results[2 * b + 1]["out"])
        for s in range(T // STRIPE):
            out[b, s * STRIPE:s * STRIPE + hr] = r0[s * hr:(s + 1) * hr]
            out[b, s * STRIPE + hr:(s + 1) * STRIPE] = r1[s * hr:(s + 1) * hr]
    return out


def kernel(x, ve, qkv_w, lambdas, c_proj_w, c_proj_b):
    nc = _get_nc(T)
    in_maps = make_in_maps(x, ve, qkv_w, lambdas, c_proj_w, c_proj_b, T)
    r = run_bass_kernel_spmd(nc, in_maps, list(range(N_CORES)))
    return assemble(r.results)
